# revision 26
# baseline (speedup 1.0000x reference)
"""Trainium2 Bass kernel for Enformer-style relative-position attention.

Problem: b=2, n=1536, dim=1536, 8 heads, dk=64, dv=192, rel-pos features=192.

Sharding: pure sequence sharding, no collectives. 8 cores = 2 batches x 4
query-row slices of 384. Each core computes full k/v for its batch
(duplicated within the 4-core batch group), attention + output projection
for its 384 query rows. Host concatenates the 8 (384, 1536) slices.

relative_shift is realized as a DRAM round trip: the pre-shift band
S_pre (128 x 1663) is written row-major to a flat DRAM scratch, and read
back with row stride 1662 starting at offset 127, which lands
shifted[p, j] = S_pre[p, 127 - p + j] exactly.
"""

import math
import os

import ml_dtypes
import numpy as np

import concourse.bass as bass
import concourse.mybir as mybir
import concourse.tile as tile
from concourse import bacc
from concourse.bass_utils import run_bass_kernel_spmd
from concourse.masks import make_identity
from concourse.tile_rust import add_dep_helper

BF16 = ml_dtypes.bfloat16
F32 = mybir.dt.float32
F32R = mybir.dt.float32r
BF = mybir.dt.bfloat16
AF = mybir.ActivationFunctionType

P = 128
N = 1536          # sequence length
D = 1536          # model dim
H = 8             # heads
DK = 64           # key dim per head
DV = 192          # value dim per head
NB = 384          # query rows per core
NT = NB // P      # q-tiles per core = 3
KC = D // P       # contraction chunks over model dim = 12
WB = N + P - 1    # pre-shift band width = 1663
RBS = WB - 1      # readback row stride = 1662
FLAT = P * WB     # flat scratch elements per (tile, head) = 212864
NPOS = 2 * N - 1  # 3071 relative positions
NRF = 192         # rel-pos feature size


def _np_positions():
    """numpy replication of reference.get_positional_embed(1536, 192)."""
    from scipy.special import gammaln as sp_gammaln

    n, feature_size = N, NRF
    dist = np.arange(-n + 1, n)
    adist = np.abs(dist).astype(np.float64)[:, None]
    num_basis = feature_size // 6
    max_range = math.log(n) / math.log(2.0)
    half_life = 2.0 ** np.linspace(3.0, max_range, num_basis)
    feat_exp = np.exp(-math.log(2.0) / half_life[None, :] * adist)
    center_widths = 2.0 ** np.arange(1, num_basis + 1) - 1.0
    feat_cm = (center_widths[None, :] > adist).astype(np.float64)
    stddev = n / (2 * num_basis)
    start_mean = n / num_basis
    mean = np.linspace(start_mean, float(n), num_basis)[None, :]
    concentration = (mean / stddev) ** 2
    rate = mean / (stddev**2)
    with np.errstate(divide="ignore", invalid="ignore"):
        xl = np.where(
            adist > 0,
            (concentration - 1.0) * np.log(np.where(adist > 0, adist, 1.0)),
            0.0,
        )
        xl = np.where((adist == 0) & (concentration - 1.0 != 0), -np.inf, xl)
    log_unnorm = xl - rate * adist
    log_norm = sp_gammaln(concentration) - concentration * np.log(rate)
    prob = np.exp(log_unnorm - log_norm) + 1e-8
    feat_gamma = prob / np.max(prob, axis=-1, keepdims=True)
    emb = np.concatenate([feat_exp, feat_cm, feat_gamma], axis=-1)
    sign = np.sign(dist).astype(np.float64)[:, None]
    return np.concatenate([emb, sign * emb], axis=-1).astype(np.float32)


def _r(ap):
    return ap.bitcast(F32R)


def _build_nc():
    nc = bacc.Bacc("TRN2", target_bir_lowering=False)

    xT = nc.dram_tensor("xT", [D, N], F32R, kind="ExternalInput")
    xTq = nc.dram_tensor("xTq", [D, NB], F32R, kind="ExternalInput")
    xb = nc.dram_tensor("xb", [KC, P, KC * P], mybir.dt.bfloat16, kind="ExternalInput")
    wqT = nc.dram_tensor("wqT", [D, H * DK], F32R, kind="ExternalInput")
    wkT = nc.dram_tensor("wkT", [D, H * DK], F32R, kind="ExternalInput")
    wvT = nc.dram_tensor("wvT", [D, H * DV], mybir.dt.bfloat16, kind="ExternalInput")
    woT = nc.dram_tensor("woT", [H * DV, D], mybir.dt.bfloat16, kind="ExternalInput")
    wrTa = nc.dram_tensor("wrTa", [P, H * DK], mybir.dt.bfloat16, kind="ExternalInput")
    wrTb = nc.dram_tensor("wrTb", [NRF - P, H * DK], mybir.dt.bfloat16, kind="ExternalInput")
    # positions^T window for this core (covers all 3 tiles' bands), split
    # over the 192-feature axis. Window width = 1663 + 2*128 = 1919, pad 1920.
    posa = nc.dram_tensor("posa", [P, 1920], mybir.dt.bfloat16, kind="ExternalInput")
    posb = nc.dram_tensor("posb", [NRF - P, 1920], mybir.dt.bfloat16, kind="ExternalInput")
    rcb = nc.dram_tensor("rcb", [P, 4], F32, kind="ExternalInput")
    rpb = nc.dram_tensor("rpb", [P, 4], F32, kind="ExternalInput")
    bo = nc.dram_tensor("bo", [1, D], mybir.dt.bfloat16, kind="ExternalInput")

    out = nc.dram_tensor("out", [NB, D], F32, kind="ExternalOutput")

    scale = DK ** -0.5

    with tile.TileContext(nc) as tc:
        with (
            tc.tile_pool(name="res", bufs=1) as res,
            tc.tile_pool(name="psum_mm", bufs=6, space="PSUM") as pmm,
            tc.tile_pool(name="psum_av", bufs=2, space="PSUM") as pav,
            tc.tile_pool(name="dram", bufs=6, space="DRAM") as dsc,
        ):
            # long-lived SBUF residents
            kT_sb = res.tile([P, 4 * N], F32R, tag="kT")          # 24 KB/part
            v_sb = res.tile([P, KC * H * DV], BF, tag="v")       # 36 KB/part
            qcT_sb = res.tile([P, 4 * NB], F32R, tag="qcT")       # 6 KB/part
            qpT_sb = res.tile([P, 4 * NB], BF, tag="qpT")        # 3 KB/part
            rcb_sb = res.tile([P, 4], F32, tag="rcb")
            rpb_sb = res.tile([P, 4], F32, tag="rpb")
            bo_sb = res.tile([1, D], BF, tag="bo")
            ones_sb = res.tile([1, P], BF, tag="ones")

            nc.scalar.dma_start(out=rcb_sb[:], in_=rcb[:])
            nc.scalar.dma_start(out=rpb_sb[:], in_=rpb[:])
            nc.scalar.dma_start(out=bo_sb[:], in_=bo[:])
            wra_sb = res.tile([P, H * DK], BF, tag="wra")
            wrb_sb = res.tile([NRF - P, H * DK], BF, tag="wrb")
            wo_sb = res.tile([P, KC * D], BF, tag="wo")      # 18 KB/part
            posa_sb = res.tile([P, 1920], BF, tag="posa")
            posb_sb = res.tile([NRF - P, 1920], BF, tag="posb")

            nc.vector.memset(ones_sb[:], 1.0)
            nc.scalar.dma_start(out=wra_sb[:], in_=wrTa[:])
            nc.scalar.dma_start(out=wrb_sb[:], in_=wrTb[:])
            nc.scalar.dma_start(
                out=wo_sb[:].rearrange("p (cc c) -> p cc c", c=D),
                in_=woT[:].rearrange("(cc p) c -> p cc c", p=P),
            )
            nc.scalar.dma_start(out=posa_sb[:], in_=posa[:])
            nc.scalar.dma_start(out=posb_sb[:], in_=posb[:])

            # ---------------- k projection: kT = Wk @ x^T ----------------
            # kT_sb layout: M-tile m (128 rows of h*dk) at cols [m*N, (m+1)*N)
            with (
                tc.tile_pool(name="xs", bufs=3) as xs,
                tc.tile_pool(name="ws", bufs=1) as ws,
            ):
                wk_sb = ws.tile([P, KC * 512], F32R, tag="wk")
                nc.sync.dma_start(
                    out=wk_sb[:].rearrange("p (kc c) -> p kc c", c=512),
                    in_=wkT[:].rearrange("(kc p) c -> p kc c", p=P),
                )
                for nc3 in range(3):  # N-chunks of 512 over sequence
                    for kc in range(KC):
                        xt = xs.tile([P, 512], F32R, tag="xs")
                        nc.sync.dma_start(
                            out=xt[:], in_=xT[kc * P : (kc + 1) * P, nc3 * 512 : (nc3 + 1) * 512]
                        )
                        if kc == 0:
                            pk = [pmm.tile([P, 512], F32, tag="pmm", name=f"pk{m_}") for m_ in range(4)]
                        for m in range(4):
                            nc.tensor.matmul(
                                pk[m][:],
                                wk_sb[:, kc * 512 + m * P : kc * 512 + (m + 1) * P],
                                xt[:],
                                start=(kc == 0),
                                stop=(kc == KC - 1),
                            )
                    for m in range(4):
                        nc.scalar.copy(kT_sb[:, m * N + nc3 * 512 : m * N + (nc3 + 1) * 512], pk[m][:])

                # ---------------- q projection (own 384 rows) ----------------
                # xTq chunks resident for the pass; psum per M-tile accumulated
                # over kc; evicted twice: qcT = psum*scale + rcb ; qpT likewise
                with tc.tile_pool(name="xq", bufs=1) as xqp:
                    xq_sb = xqp.tile([P, KC * NB], F32R, tag="xq")
                    wq_sb = xqp.tile([P, KC * 512], F32R, tag="wq")
                    nc.scalar.dma_start(
                        out=xq_sb[:].rearrange("p (kc c) -> p kc c", c=NB),
                        in_=xTq[:].rearrange("(kc p) c -> p kc c", p=P),
                    )
                    nc.scalar.dma_start(
                        out=wq_sb[:].rearrange("p (kc c) -> p kc c", c=512),
                        in_=wqT[:].rearrange("(kc p) c -> p kc c", p=P),
                    )
                    for m in range(4):
                        pq = pmm.tile([P, NB], F32, tag="pmm")
                        for kc in range(KC):
                            nc.tensor.matmul(
                                pq[:], wq_sb[:, kc * 512 + m * P : kc * 512 + (m + 1) * P],
                                xq_sb[:, kc * NB : (kc + 1) * NB],
                                start=(kc == 0), stop=(kc == KC - 1),
                            )
                        nc.scalar.activation(
                            qcT_sb[:, m * NB : (m + 1) * NB], pq[:], AF.Identity,
                            bias=rcb_sb[:, m : m + 1], scale=scale,
                        )
                        nc.scalar.activation(
                            qpT_sb[:, m * NB : (m + 1) * NB], pq[:], AF.Identity,
                            bias=rpb_sb[:, m : m + 1], scale=scale,
                        )

            # ---------------- v projection: v = x @ Wv^T (bf16) ----------------
            # v_sb layout: key-chunk kc at cols [kc*H*DV, (kc+1)*H*DV)
            with (
                tc.tile_pool(name="wv", bufs=1) as wvp,
                tc.tile_pool(name="xbs", bufs=3) as xbs,
            ):
                wv_sb = wvp.tile([P, KC * H * DV], BF, tag="wv")  # 36 KB/part
                nc.scalar.dma_start(
                    out=wv_sb[:].rearrange("p (kc c) -> p kc c", c=H * DV),
                    in_=wvT[:].rearrange("(kc p) c -> p kc c", p=P),
                )
                for m in range(KC):  # output row-chunk (keys)
                    pv = [pmm.tile([P, 512], F32, tag="pmm", name=f"pv{n_}") for n_ in range(3)]
                    xbt = xbs.tile([P, KC * P], BF, tag="xbs")
                    nc.sync.dma_start(out=xbt[:], in_=xb[m])
                    for kc in range(KC):
                        for nc3 in range(3):
                            nc.tensor.matmul(
                                pv[nc3][:],
                                xbt[:, kc * P : (kc + 1) * P],
                                wv_sb[:, kc * H * DV + nc3 * 512 : kc * H * DV + (nc3 + 1) * 512],
                                start=(kc == 0),
                                stop=(kc == KC - 1),
                            )
                    for nc3 in range(3):
                        nc.scalar.copy(
                            v_sb[:, m * H * DV + nc3 * 512 : m * H * DV + (nc3 + 1) * 512],
                            pv[nc3][:],
                        )

            # ---------------- attention + output, per q-tile ----------------
            with (
                tc.tile_pool(name="band", bufs=2) as bandp,
                tc.tile_pool(name="stg", bufs=3) as stgp,
                tc.tile_pool(name="rb", bufs=3) as rbp,
                tc.tile_pool(name="es", bufs=2) as esp,
                tc.tile_pool(name="at", bufs=2) as atp,
                tc.tile_pool(name="av", bufs=2) as avp,
                tc.tile_pool(name="avT", bufs=2) as avTp,
                tc.tile_pool(name="ou", bufs=1) as oup,
                tc.tile_pool(name="sm", bufs=4) as smp,
            ):
                # rel_k^T over the core's 1920-wide position window, computed
                # once per head-pair into DRAM; per-tile bands are slices of it
                rel_dram = []
                for hh in range(4):
                    rk = bandp.tile([P, 1920], BF, tag="rk", name=f"rk{hh}")
                    for nc4 in range(4):
                        prk = pmm.tile([P, 480], F32, tag="pmm", name=f"prk{hh}_{nc4}")
                        nc.tensor.matmul(
                            prk[:],
                            wra_sb[:, hh * P : (hh + 1) * P],
                            posa_sb[:, nc4 * 480 : (nc4 + 1) * 480],
                            start=True, stop=False,
                        )
                        nc.tensor.matmul(
                            prk[:],
                            wrb_sb[:, hh * P : (hh + 1) * P],
                            posb_sb[:, nc4 * 480 : (nc4 + 1) * 480],
                            start=False, stop=True,
                        )
                        nc.vector.tensor_copy(rk[:, nc4 * 480 : (nc4 + 1) * 480], prk[:])
                    rd = dsc.tile([P, 1920], BF, tag="rd", name=f"rd{hh}", bufs=4)
                    nc.scalar.dma_start(out=rd[:], in_=rk[:])
                    rel_dram.append(rd)

                for t in range(NT):
                    attnv_sb = avp.tile([P, H * DV], BF, tag="attnv")
                    for h in range(H):
                        hh = h // 2
                        ho = (h % 2) * DK
                        if h % 2 == 0:
                            band_sb = bandp.tile([P, WB + 1], BF, tag="band")
                            nc.scalar.dma_start(
                                out=band_sb[:, :WB],
                                in_=rel_dram[hh][:, 256 - 128 * t : 256 - 128 * t + WB],
                            )

                        # pre-shift rel logits S_pre (128, 1663) -> DRAM (bf16)
                        stg = stgp.tile([P, WB + 1], BF, tag="stg")
                        for nc4 in range(4):
                            w = min(512, WB - nc4 * 512)
                            pp = pmm.tile([P, 512], F32, tag="pmm")
                            nc.tensor.matmul(
                                pp[:, :w],
                                qpT_sb[ho : ho + DK, hh * NB + t * P : hh * NB + (t + 1) * P],
                                band_sb[ho : ho + DK, nc4 * 512 : nc4 * 512 + w],
                                start=True, stop=True,
                            )
                            nc.scalar.copy(stg[:, nc4 * 512 : nc4 * 512 + w], pp[:, :w])
                        slot = dsc.tile([FLAT], BF, tag="slot")
                        slot_pw = slot[:].rearrange("(p w) -> p w", w=WB)
                        for nc4 in range(4):
                            w = min(512, WB - nc4 * 512)
                            nc.gpsimd.dma_start(
                                out=slot_pw[:, nc4 * 512 : nc4 * 512 + w],
                                in_=stg[:, nc4 * 512 : nc4 * 512 + w],
                            )
                        # shifted readback: flat[127 + p*1662 + j]
                        rbt = rbp.tile([P, N], BF, tag="rb")
                        rb_dma = nc.gpsimd.dma_start(
                            out=rbt[:],
                            in_=slot[P - 1 : P - 1 + P * RBS].rearrange(
                                "(p w) -> p w", w=RBS
                            )[:, :N],
                        )

                        # content logits -> SBUF fast (short PSUM hold), then
                        # one full-width rel add and one full-width exp
                        es = esp.tile([P, N], BF, tag="es")
                        cont = esp.tile([P, N], F32, tag="cont", bufs=3)
                        sums = smp.tile([P, 1], F32, tag="sums")
                        for nc3 in range(3):
                            ps = pmm.tile([P, 512], F32, tag="pmm")
                            nc.tensor.matmul(
                                ps[:],
                                qcT_sb[ho : ho + DK, hh * NB + t * P : hh * NB + (t + 1) * P],
                                kT_sb[ho : ho + DK, hh * N + nc3 * 512 : hh * N + (nc3 + 1) * 512],
                                start=True, stop=True,
                            )
                            nc.vector.tensor_copy(cont[:, nc3 * 512 : (nc3 + 1) * 512], ps[:])
                        nc.vector.tensor_add(cont[:], cont[:], rbt[:])
                        nc.scalar.activation(
                            es[:], cont[:], AF.Exp, accum_out=sums[:]
                        )
                        recip = smp.tile([P, 1], F32, tag="recip")
                        nc.vector.reciprocal(recip[:], sums[:])

                        # attnv = exp(S) @ v; exp(S)^T via one batched XBAR
                        # DMA transpose: att_all[:, kc, :] = es[:, kc-block]^T
                        att_all = atp.tile([P, KC * P], BF, tag="at")
                        nc.sync.dma_start_transpose(
                            att_all[:].rearrange("p (kc c) -> p kc c", c=P), es[:]
                        )
                        pv = pav.tile([P, DV], F32, tag="pav")
                        for kc in range(KC):
                            nc.tensor.matmul(
                                pv[:],
                                att_all[:, kc * P : (kc + 1) * P],
                                v_sb[:, kc * H * DV + h * DV : kc * H * DV + (h + 1) * DV],
                                start=(kc == 0),
                                stop=(kc == KC - 1),
                            )
                        nc.scalar.activation(
                            attnv_sb[:, h * DV : (h + 1) * DV], pv[:], AF.Copy,
                            scale=recip[:],
                        )

                    # ---- output projection for tile t: out = attnv @ Wout^T + b ----
                    avT_all = avTp.tile([P, KC * P], BF, tag="avT")
                    nc.sync.dma_start_transpose(
                        avT_all[:].rearrange("p (cc c) -> p cc c", c=P), attnv_sb[:]
                    )
                    po = [pmm.tile([P, 512], F32, tag="pmm", name=f"po{n_}") for n_ in range(3)]
                    for cc in range(KC):
                        for nc3 in range(3):
                            nc.tensor.matmul(
                                po[nc3][:],
                                avT_all[:, cc * P : (cc + 1) * P],
                                wo_sb[:, cc * D + nc3 * 512 : cc * D + (nc3 + 1) * 512],
                                start=(cc == 0),
                                stop=False,
                            )
                    ot = oup.tile([P, D], F32, tag="ou")
                    for nc3 in range(3):
                        nc.tensor.matmul(
                            po[nc3][:],
                            ones_sb[:],
                            bo_sb[:, nc3 * 512 : (nc3 + 1) * 512],
                            start=False,
                            stop=True,
                        )
                        nc.scalar.copy(ot[:, nc3 * 512 : (nc3 + 1) * 512], po[nc3][:])
                    nc.sync.dma_start(out=out[t * P : (t + 1) * P, :], in_=ot[:])

    nc.compile()
    return nc


_CACHE = {}


def _get_nc():
    if "nc" not in _CACHE:
        _CACHE["nc"] = _build_nc()
    return _CACHE["nc"]


def kernel(x, Wq, Wk, Wv, Wrel, Wout, b_out, rel_content_bias, rel_pos_bias):
    x = np.asarray(x, np.float32)
    Wq = np.asarray(Wq, np.float32)
    Wk = np.asarray(Wk, np.float32)
    Wv = np.asarray(Wv, np.float32)
    Wrel = np.asarray(Wrel, np.float32)
    Wout = np.asarray(Wout, np.float32)
    b_out = np.asarray(b_out, np.float32)
    rcb = np.asarray(rel_content_bias, np.float32).reshape(H * DK)
    rpb = np.asarray(rel_pos_bias, np.float32).reshape(H * DK)

    positions = _np_positions()  # (3071, 192) f32, input-independent constant
    posT = np.ascontiguousarray(positions.T).astype(BF16)  # (192, 3071)

    wqT = np.ascontiguousarray(Wq.T)
    wkT = np.ascontiguousarray(Wk.T)
    wvT = np.ascontiguousarray(Wv.T).astype(BF16)
    woT = np.ascontiguousarray(Wout.T).astype(BF16)
    wrT = np.ascontiguousarray(Wrel.T).astype(BF16)  # (192, 512)
    rcb_in = np.ascontiguousarray(rcb.reshape(4, P).T)  # (128, 4)
    rpb_in = np.ascontiguousarray(rpb.reshape(4, P).T)
    bo_in = b_out.reshape(1, D).astype(BF16)

    in_maps = []
    for core in range(8):
        bi, ci = core // 4, core % 4
        q0 = ci * NB
        xTb = np.ascontiguousarray(x[bi].T)  # (dim, n) f32
        # positions window covering all 3 tiles' bands: [lo_2, lo_0 + WB)
        base = N - 1 - (q0 + 2 * P) - (P - 1)
        wnd = np.zeros((NRF, 1920), BF16)
        wnd[:, : WB + 256] = posT[:, base : base + WB + 256]
        in_maps.append(
            {
                "xT": xTb,
                "xTq": np.ascontiguousarray(xTb[:, q0 : q0 + NB]),
                "xb": np.ascontiguousarray(
                    xTb.astype(BF16).reshape(12, 128, 12, 128).transpose(2, 1, 0, 3).reshape(12, 128, 1536)
                ),
                "wqT": wqT,
                "wkT": wkT,
                "wvT": wvT,
                "woT": woT,
                "wrTa": np.ascontiguousarray(wrT[:P]),
                "wrTb": np.ascontiguousarray(wrT[P:]),
                "posa": np.ascontiguousarray(wnd[:P]),
                "posb": np.ascontiguousarray(wnd[P:]),
                "rcb": rcb_in,
                "rpb": rpb_in,
                "bo": bo_in,
            }
        )

    nc = _get_nc()
    trace = bool(os.environ.get("KERNEL_TRACE"))
    res = run_bass_kernel_spmd(nc, in_maps, list(range(8)), trace=trace)
    _CACHE["last_res"] = res

    out = np.empty((2, N, D), np.float32)
    for core in range(8):
        bi, ci = core // 4, core % 4
        out[bi, ci * NB : (ci + 1) * NB] = res.results[core]["out"]
    return out
